# revision 11
# baseline (speedup 1.0000x reference)
"""Trainium2 Bass kernel for the RetinaNet-style detection post-process.

reference semantics (per anchor row r of B*N rows):
    score[r] = max_c cls[r, c]          (C = 500)
    idx[r]   = argmax_c cls[r, c]       (first occurrence)
    mask[r]  = score[r] > cls_thresh
    boxes[r] = clip(decode(anchors[r], regressions[r], regress_factor)) * mask
    score/idx zeroed where ~mask

Strategy (per core; 8 cores, data-parallel over 50k contiguous rows each):
  - rows live on SBUF partitions, 10 rows per partition per tile (1280
    rows / 2.56 MB per class DMA, fully contiguous).
  - per row: VectorE tensor_tensor_scan(op0=max, op1=bypass) computes the
    prefix-max M; M[-1] is the row max (exact, one DVE pass).
  - argmax = #{c : M[c] < max}. Because M is non-decreasing, that count is
    exactly the first index attaining the max (jnp.argmax tie semantics).
    The count is Sign(max - M) summed via the ScalarE activation
    accumulator (one ACT pass; Sign/Exp/Copy share one activation table
    set, so no table reloads). A tunable subset of rows instead uses
    VectorE tensor_scalar(is_lt)+accum to balance DVE vs ACT load.
  - box decode/clip and output masking run at supertile granularity
    (13 tiles = 16640 rows) so the [128, 130]-shaped elementwise ops
    amortize instruction overhead.
"""

import numpy as np

import concourse.bacc as bacc
import concourse.mybir as mybir
from concourse.tile import TileContext

F32 = mybir.dt.float32
I32 = mybir.dt.int32
U8 = mybir.dt.uint8
ALU = mybir.AluOpType
ACTF = mybir.ActivationFunctionType

B, N, C = 4, 100000, 500
NCORES = 8
ROWS = (B * N) // NCORES  # 50000 rows per core
P = 128
TC = 10                   # rows per partition per tile
TILE_ROWS = P * TC        # 1280
SUPER = 13                # tiles per supertile (output/decode granularity)
H = W = 512.0
# sub-rows (t index) whose argmax count runs on VectorE instead of ScalarE
DVE_T = {0}


def _ceil_div(a, b):
    return (a + b - 1) // b


def _emit_supertile(nc, tc, pools, io, base, tile_ps):
    """Emit one supertile: tiles covering rows [base, base+sum(p*TC)).

    tile_ps: list of partition counts, one per tile (128 for full tiles).
    """
    (pool_x, pool_m, pool_scr, pool_sup) = pools
    (cls_in, reg_in, anc_in, thr_t, rf_t,
     boxes_out, idx_out, scores_out, mask_out) = io

    n_tiles = len(tile_ps)
    pmax = max(tile_ps)
    cols = n_tiles * TC
    srows = sum(p * TC for p in tile_ps)

    idxf = pool_sup.tile([pmax, cols], F32, tag="idxf")
    mx = pool_sup.tile([pmax, cols], F32, tag="mx")

    tbase = base
    for g, tp in enumerate(tile_ps):
        x = pool_x.tile([tp, TC * C], F32, tag="x")
        src = cls_in[tbase : tbase + tp * TC, :].rearrange(
            "(p t) c -> p (t c)", p=tp
        )
        nc.sync.dma_start(x[:], src)
        m = pool_m.tile([tp, TC * C], F32, tag="m")
        for t in range(TC):
            sl = slice(t * C, (t + 1) * C)
            nc.vector.tensor_tensor_scan(
                m[:, sl], x[:, sl], x[:, sl], 0.0,
                op0=ALU.max, op1=ALU.bypass,
            )
        # gather the 10 row-maxima (strided: every C-th element, offset C-1)
        mv = m[:].rearrange("p (t c) -> p t c", c=C)[:, :, C - 1]
        nc.vector.tensor_copy(mx[:tp, g * TC : (g + 1) * TC], mv)
        for t in range(TC):
            sl = slice(t * C, (t + 1) * C)
            m_last = m[:, (t + 1) * C - 1 : (t + 1) * C]
            acc = idxf[:tp, g * TC + t : g * TC + t + 1]
            if t in DVE_T:
                lt = pool_scr.tile([tp, C], F32, tag="lt")
                nc.vector.tensor_scalar(
                    lt[:], m[:, sl], m_last, None,
                    op0=ALU.is_lt, op1=ALU.add, accum_out=acc,
                )
            else:
                sgn = pool_scr.tile([tp, C], F32, tag="sgn")
                nc.scalar.activation(
                    sgn[:], m[:, sl], ACTF.Sign,
                    bias=m_last, scale=-1.0, accum_out=acc,
                )
        tbase += tp * TC

    # ---- supertile-level score/mask/index assembly ----
    maskf = pool_sup.tile([pmax, cols], F32, tag="maskf")
    nc.vector.tensor_scalar(maskf[:], mx[:], thr_t[:pmax, 0:1], None, op0=ALU.is_gt)
    mask_u8 = pool_sup.tile([pmax, cols], U8, tag="mask_u8")
    nc.vector.tensor_copy(mask_u8[:], maskf[:])
    scores = pool_sup.tile([pmax, cols], F32, tag="scores")
    nc.vector.tensor_tensor(scores[:], mx[:], maskf[:], op=ALU.mult)
    idxm = pool_sup.tile([pmax, cols], F32, tag="idxm")
    nc.vector.tensor_tensor(idxm[:], idxf[:], maskf[:], op=ALU.mult)
    idx_i32 = pool_sup.tile([pmax, cols], I32, tag="idx_i32")
    nc.vector.tensor_copy(idx_i32[:], idxm[:])

    # ---- box decode (all [pmax, cols] elementwise) ----
    # DMA anchors/regressions with the same (p, g, t) row mapping.
    assert len(set(tile_ps)) == 1
    tp0 = tile_ps[0]
    rearr = lambda ap: ap.rearrange("(g p t) c -> p g t c", g=n_tiles, p=tp0)
    # sbuf-side 4D view matching [p, g, t, c]
    sb4 = lambda tile_ap: tile_ap.rearrange("p (g t c) -> p g t c", g=n_tiles, c=4)
    anc_t = pool_sup.tile([pmax, cols * 4], F32, tag="anc_t")
    nc.sync.dma_start(sb4(anc_t[:]), rearr(anc_in[base : base + srows, :]))
    reg_t = pool_sup.tile([pmax, cols * 4], F32, tag="reg_t")
    nc.sync.dma_start(sb4(reg_t[:]), rearr(reg_in[base : base + srows, :]))

    av = anc_t[:].rearrange("p (r c) -> p r c", c=4)
    rv = reg_t[:].rearrange("p (r c) -> p r c", c=4)

    def sup(tag):
        return pool_sup.tile([pmax, cols], F32, tag=tag, name=tag)

    wdt, hgt = sup("wdt"), sup("hgt")
    nc.vector.tensor_tensor(wdt[:], av[:, :, 2], av[:, :, 0], op=ALU.subtract)
    nc.vector.tensor_tensor(hgt[:], av[:, :, 3], av[:, :, 1], op=ALU.subtract)
    cx, cy = sup("cx"), sup("cy")
    nc.vector.scalar_tensor_tensor(
        cx[:], wdt[:], 0.5, av[:, :, 0], op0=ALU.mult, op1=ALU.add)
    nc.vector.scalar_tensor_tensor(
        cy[:], hgt[:], 0.5, av[:, :, 1], op0=ALU.mult, op1=ALU.add)
    dxw, dyh = sup("dxw"), sup("dyh")
    nc.vector.scalar_tensor_tensor(
        dxw[:], rv[:, :, 0], rf_t[:pmax, 0:1], wdt[:], op0=ALU.mult, op1=ALU.mult)
    nc.vector.scalar_tensor_tensor(
        dyh[:], rv[:, :, 1], rf_t[:pmax, 1:2], hgt[:], op0=ALU.mult, op1=ALU.mult)
    pcx, pcy = sup("pcx"), sup("pcy")
    nc.vector.tensor_tensor(pcx[:], cx[:], dxw[:], op=ALU.add)
    nc.vector.tensor_tensor(pcy[:], cy[:], dyh[:], op=ALU.add)
    dw, dh = sup("dw"), sup("dh")
    nc.vector.tensor_scalar(dw[:], rv[:, :, 2], rf_t[:pmax, 2:3], None, op0=ALU.mult)
    nc.vector.tensor_scalar(dh[:], rv[:, :, 3], rf_t[:pmax, 3:4], None, op0=ALU.mult)
    edw, edh = sup("edw"), sup("edh")
    nc.scalar.activation(edw[:], dw[:], ACTF.Exp)
    nc.scalar.activation(edh[:], dh[:], ACTF.Exp)
    hw_, hh_ = sup("hw_"), sup("hh_")
    nc.vector.scalar_tensor_tensor(
        hw_[:], edw[:], 0.5, wdt[:], op0=ALU.mult, op1=ALU.mult)
    nc.vector.scalar_tensor_tensor(
        hh_[:], edh[:], 0.5, hgt[:], op0=ALU.mult, op1=ALU.mult)

    boxes_t = pool_sup.tile([pmax, cols * 4], F32, tag="boxes_t")
    bv = boxes_t[:].rearrange("p (r c) -> p r c", c=4)
    crd = sup("crd")
    for k, (ctr, half, op, hi) in enumerate((
        (pcx, hw_, ALU.subtract, W),   # x1
        (pcy, hh_, ALU.subtract, H),   # y1
        (pcx, hw_, ALU.add, W),        # x2
        (pcy, hh_, ALU.add, H),        # y2
    )):
        nc.vector.tensor_tensor(crd[:], ctr[:], half[:], op=op)
        nc.vector.tensor_scalar(crd[:], crd[:], 0.0, hi, op0=ALU.max, op1=ALU.min)
        nc.vector.tensor_tensor(bv[:, :, k], crd[:], maskf[:], op=ALU.mult)

    # ---- outputs ----
    rearr1 = lambda ap: ap.rearrange("(g p t) -> p g t", g=n_tiles, p=tp0)
    sb3 = lambda tile_ap: tile_ap.rearrange("p (g t) -> p g t", g=n_tiles)
    nc.sync.dma_start(rearr(boxes_out[base : base + srows, :]), sb4(boxes_t[:]))
    nc.sync.dma_start(rearr1(scores_out[base : base + srows]), sb3(scores[:]))
    nc.sync.dma_start(rearr1(idx_out[base : base + srows]), sb3(idx_i32[:]))
    nc.sync.dma_start(rearr1(mask_out[base : base + srows]), sb3(mask_u8[:]))


def build_program(rows=ROWS):
    """Build the per-core Bass program for `rows` anchor rows."""
    assert rows % TC == 0
    prows = rows // TC                      # partition-rows
    n_full = prows // P                     # full 128-partition tiles
    tail_p = prows - n_full * P             # partial-tile partition count

    nc = bacc.Bacc("TRN2", target_bir_lowering=False, debug=False)
    cls_in = nc.dram_tensor("cls", [rows, C], F32, kind="ExternalInput").ap()
    reg_in = nc.dram_tensor("reg", [rows, 4], F32, kind="ExternalInput").ap()
    anc_in = nc.dram_tensor("anc", [rows, 4], F32, kind="ExternalInput").ap()
    thr_in = nc.dram_tensor("thr", [1], F32, kind="ExternalInput").ap()
    rf_in = nc.dram_tensor("rf", [4], F32, kind="ExternalInput").ap()
    boxes_out = nc.dram_tensor("boxes", [rows, 4], F32, kind="ExternalOutput").ap()
    idx_out = nc.dram_tensor("idx", [rows], I32, kind="ExternalOutput").ap()
    scores_out = nc.dram_tensor("scores", [rows], F32, kind="ExternalOutput").ap()
    mask_out = nc.dram_tensor("mask", [rows], U8, kind="ExternalOutput").ap()

    with TileContext(nc) as tc:
        with (
            tc.tile_pool(name="x", bufs=3) as pool_x,
            tc.tile_pool(name="m", bufs=2) as pool_m,
            tc.tile_pool(name="scr", bufs=2) as pool_scr,
            tc.tile_pool(name="sup", bufs=2) as pool_sup,
            tc.tile_pool(name="cst", bufs=1) as pool_cst,
        ):
            # broadcast runtime scalars to all partitions once
            thr_s = pool_cst.tile([1, 1], F32, tag="thr_s")
            nc.sync.dma_start(thr_s[:], thr_in[None, :])
            thr_t = pool_cst.tile([P, 1], F32, tag="thr")
            nc.gpsimd.partition_broadcast(thr_t[:], thr_s[:])
            rf_s = pool_cst.tile([1, 4], F32, tag="rf_s")
            nc.sync.dma_start(rf_s[:], rf_in[None, :])
            rf_t = pool_cst.tile([P, 4], F32, tag="rf")
            nc.gpsimd.partition_broadcast(rf_t[:], rf_s[:])

            pools = (pool_x, pool_m, pool_scr, pool_sup)
            io = (cls_in, reg_in, anc_in, thr_t, rf_t,
                  boxes_out, idx_out, scores_out, mask_out)

            base = 0
            for s in range(_ceil_div(n_full, SUPER)):
                k = min(SUPER, n_full - s * SUPER)
                tile_ps = [P] * k
                _emit_supertile(nc, tc, pools, io, base, tile_ps)
                base += k * TILE_ROWS
            if tail_p:
                _emit_supertile(nc, tc, pools, io, base, [tail_p])
    nc.compile()
    return nc


_PROG_CACHE = {}


def _get_program(rows=ROWS):
    if rows not in _PROG_CACHE:
        _PROG_CACHE[rows] = build_program(rows)
    return _PROG_CACHE[rows]


def run_cores(per_core_inputs, rows=ROWS, **run_kwargs):
    """Run the program on len(per_core_inputs) cores; returns out_maps."""
    from concourse.bass_utils import run_bass_kernel_spmd

    nc = _get_program(rows)
    res = run_bass_kernel_spmd(
        nc, per_core_inputs, core_ids=list(range(len(per_core_inputs))),
        **run_kwargs,
    )
    return res


def make_in_maps(inputs):
    cls = np.ascontiguousarray(
        np.asarray(inputs["classifications"], dtype=np.float32).reshape(B * N, C))
    reg = np.ascontiguousarray(
        np.asarray(inputs["regressions"], dtype=np.float32).reshape(B * N, 4))
    anc1 = np.asarray(inputs["anchors"], dtype=np.float32).reshape(N, 4)
    thr = np.asarray(inputs["cls_thresh"], dtype=np.float32).reshape(1)
    rf = np.asarray(inputs["regress_factor"], dtype=np.float32).reshape(4)

    in_maps = []
    for k in range(NCORES):
        lo = k * ROWS
        hi = lo + ROWS
        # anchors are broadcast over batch: row r -> n = r % N
        nlo = lo % N
        anc_k = np.ascontiguousarray(anc1[nlo : nlo + ROWS])
        in_maps.append({
            "cls": cls[lo:hi],
            "reg": np.ascontiguousarray(reg[lo:hi]),
            "anc": anc_k,
            "thr": thr,
            "rf": rf,
        })
    return in_maps


def kernel(imgs, classifications, regressions, anchors, cls_thresh,
           regress_factor):
    in_maps = make_in_maps(dict(
        classifications=classifications, regressions=regressions,
        anchors=anchors, cls_thresh=cls_thresh, regress_factor=regress_factor))

    res = run_cores(in_maps)
    outs = res.results
    boxes = np.concatenate([o["boxes"] for o in outs]).reshape(B, N, 4)
    cls_idx = np.concatenate([o["idx"] for o in outs]).reshape(B, N)
    scores = np.concatenate([o["scores"] for o in outs]).reshape(B, N)
    mask = np.concatenate([o["mask"] for o in outs]).reshape(B, N).astype(bool)
    return boxes, cls_idx.astype(np.int32), scores, mask
